# revision 8
# baseline (speedup 1.0000x reference)
"""Trainium2 Bass kernel for nn_FComb_79319456023150 (dense_cnn), v3.

Per-pixel MLP over a 96^3 volume: four 1x1x1 convs (38->32->32->32->1 channels
with relu between). z is batch-constant, so w1[:, 32:38] @ z folds into the
layer-1 bias and every layer becomes a K=32 channel GEMM.

Sharding: spatial (outermost X axis) across 8 cores, 110592 pixels each.
Weights/biases replicated.

Device layout per core: [128, 27648] = 4 pixel-blocks x 32 channels on
partitions, pixels on the free dim, bf16. Each layer is a BLOCK-DIAGONAL
[128, 128] matmul (4 copies of W^T), one full-array bf16 matmul per 512-col
sc applies the 32x32 GEMM to 4 pixel blocks at once.

Pipeline: 7 sc streams, one fp32 PSUM bank each; bank 7 is the L4
accumulator (rows 4j+m per 27-sc evac span). Relu+bias rides the mandatory
PSUM->SBUF crossing, which only Act and DVE may perform (GPSIMD has no PSUM
port), and in steady state PE (5973ns/7-sc group), Act (~5950) and DVE
(~6080) are all ~100% committed - the schedule is a measured three-way tie,
which is why crossing cohorts stay at [128,1024] pairs (every larger-cohort
variant loses >1us to latency cascades; measured via TimelineSim sweeps).

v3 structure (vs the previous 60.3us kernel):
 - Startup: the first DMA packs [w1|w2|w3] + the fp32 bias (bitcast to bf16
   bit-columns) + the first sc of pixels into ONE transfer (one HWDGE slot
   instead of three), group 0 loads sc-by-sc with single-sc crossing cohorts
   (pipeline ramps while DMA streams), and groups 1-2 prefetch ahead of the
   large w4 transfer. First real matmul at ~3.5us (DMA-latency floor).
 - The FIRST group is 5 scs wide (headgroup=5), so the DMA-starved ramp
   group is smaller and every later group is a full 7-wide phase-5 bank
   rotation (crossing cohorts index banks via scs[c0] mod 7). Worth ~0.9us.
 - A gpsimd-memset warm tile feeds (a) a dummy activation emitted first so
   the ACT table load happens at t~0 instead of stalling the first crossing
   (-1.3us), and (b) a burst of tiny warm-up matmuls so the PE p-state ramp
   is spent on dummy work while the first input DMA is in flight (-2.4us
   measured vs warm_n=0).
 - L4 evac spans are [27, 27] so each half ships its own output DMA; the
   second evac+DMA+sem chain is the unavoidable ~3.7us tail.
"""

import sys

import numpy as np

if "/opt/trn_rl_repo" not in sys.path:
    sys.path.insert(0, "/opt/trn_rl_repo")

C = 32          # channels per layer
P = 128         # SBUF/PSUM partitions
RG = 4          # pixel blocks stacked on the partition dim (128/32)
CH = 512        # sc width = one PSUM bank of fp32
VOL = 96 * 96 * 96                   # full volume
NCORES = 8
NPIX = VOL // NCORES                 # 110592 pixels per core
FREE = NPIX // RG                    # 27648 free-dim columns per core
NSC = FREE // CH                     # 54 scs per core
NS = 7                               # parallel sc streams (PSUM banks 0-6)
WCOLS = 3 * P                        # packed [w1|w2|w3] columns in fm_ext
assert FREE % CH == 0


# Group 0 runs its L0/L1 crossings as single-sc ops (alternating engines)
# so the pipeline ramps sc-by-sc while the input DMA is still streaming;
# L2 back to steady [1024] pairs. Found by TimelineSim search.
G0ROW_DEFAULT = [
    [(0, 1, "dve"), (1, 1, "act"), (2, 1, "dve"), (3, 1, "act"),
     (4, 1, "dve"), (5, 1, "act"), (6, 1, "dve")],
    [(0, 1, "act"), (1, 1, "dve"), (2, 1, "act"), (3, 1, "dve"),
     (4, 1, "act"), (5, 1, "dve"), (6, 1, "act")],
    [(0, 2, "act"), (2, 2, "dve"), (4, 2, "act"), (6, 1, "dve")],
]

DEFAULT_CFG = {
    "pat": [
        [("act", "dve", "act", "dve"), ("dve", "act", "dve", "act"),
         ("act", "dve", "act", "dve")],
    ],
    "fin": "act",
    "headgroup": 5,
    "g0split": [1, 1, 2],
    "cohorts": [(0, 2), (2, 2), (4, 2), (6, 1)],
    "mm4slot": 1,
    "mm4plan": [0, 5, 2],
    "xbufs": 6,
    "warm_n": 20,
    "warm_cols": 48,
    "early_act": True,
    "espans": [29, 25],
    "g0rowspec": G0ROW_DEFAULT,
}


def _build_nc(npix=NPIX, cfg=None):
    import concourse.mybir as mybir
    from concourse import bacc
    from concourse.tile import TileContext

    f32 = mybir.dt.float32
    bf16 = mybir.dt.bfloat16
    Alu = mybir.AluOpType
    Act = mybir.ActivationFunctionType

    if cfg is None:
        cfg = DEFAULT_CFG
    pat = cfg["pat"]
    fin_cfg = cfg.get("fin", "act")
    cohorts = cfg.get("cohorts", [(0, 2), (2, 2), (4, 2), (6, 1)])
    rowspec = cfg.get("rowspec")

    free = npix // RG
    nsc = free // CH
    assert free % CH == 0 and nsc >= 1
    espans_cfg = cfg.get("espans", [26, 26, 2])
    if sum(espans_cfg) != nsc:
        espans_cfg = [min(NS, nsc - g) for g in range(0, nsc, NS)]
    l4span = max(espans_cfg)

    nc = bacc.Bacc()
    # fm_ext: [w1|w2|w3] block-diag weights (384 cols), the fp32 bias packed
    # as 8 bf16-bit columns, then the pixel data.
    fm = nc.dram_tensor("fm", [P, WCOLS + 8 + free], bf16, kind="ExternalInput")
    wst4 = nc.dram_tensor("wst4", [P, l4span * P], bf16, kind="ExternalInput")
    out = nc.dram_tensor("out", [npix], bf16, kind="ExternalOutput")
    XOFF = WCOLS + 8

    # out[m*free + s*CH + n] viewed [m, s, n] for per-group stores
    out_r = out.rearrange("(m s n) -> m s n", m=RG, s=nsc, n=CH)

    hg = cfg.get("headgroup")
    if hg:
        sgroups = [list(range(hg))] + [
            list(range(g, min(g + NS, nsc))) for g in range(hg, nsc, NS)]
    else:
        sgroups = [list(range(g, min(g + NS, nsc)))
                   for g in range(0, nsc, NS)]
    ts = cfg.get("tailsplit")
    if ts and len(sgroups[-1]) > max(ts):
        last = sgroups.pop()
        i = 0
        for w in ts:
            sgroups.append(last[i:i + w])
            i += w
        assert i == len(last), (ts, len(last))

    # evac spans: consecutive sc runs flushed together from bank 7
    espans = []
    s0 = 0
    for w in espans_cfg:
        espans.append(list(range(s0, s0 + w)))
        s0 += w
    assert s0 == nsc
    span_of_sc = {}
    for scs in espans:
        for j, s in enumerate(scs):
            span_of_sc[s] = (scs[0], j, len(scs), s == scs[-1])
    assert all(4 * (len(scs)) <= P for scs in espans)

    with TileContext(nc) as tc:
        with (
            tc.tile_pool(name="const", bufs=1) as constp,
            tc.tile_pool(name="data", bufs=cfg.get("xbufs", 4)) as datap,
            tc.tile_pool(name="acts", bufs=cfg.get("hbufs", 2)) as actp,
            tc.tile_pool(name="outs", bufs=2) as outsp,
            tc.tile_pool(name="psb", bufs=1, space="PSUM") as psb,
        ):
            psum = psb.tile([P, 8 * CH], f32)
            l4ps = psum[:, NS * CH:(NS + 1) * CH]

            # Warm tile: memset on the idle Pool engine at t~0; feeds the
            # early dummy activation (pulls the ACT table load off the
            # first crossing's critical path) and the PE warm-up burst.
            wcols = cfg.get("warm_cols", 48)
            warm = constp.tile([P, max(wcols, 4)], bf16)
            wscr = constp.tile([P, 4], bf16)
            if cfg.get("warm_memset_eng", "pool") == "dve":
                nc.vector.memset(warm[:, :], 0)
            else:
                nc.gpsimd.memset(warm[:, :], 0)
            if cfg.get("early_act", True):
                ea = nc.scalar.activation(
                    wscr[:, 0:2], warm[:, 0:2], Act.Relu, bias=0.0, scale=1.0)
                NAME_INFO[ea.ins.name] = (-1, "early_act")
            for wi in range(cfg.get("warm_n", 0)):
                wm = nc.tensor.matmul(
                    l4ps[:wcols, :wcols], warm[:, :wcols], warm[:, :wcols],
                    start=True, stop=True)
                NAME_INFO[wm.ins.name] = (-1, f"warm{wi}")

            # head tile: [w1|w2|w3] + packed bias + first sc in ONE DMA.
            head = constp.tile([P, XOFF + CH], bf16)
            wtile = head[:, :WCOLS]
            btile = head[:, WCOLS:XOFF].bitcast(f32)
            w4tile = constp.tile([P, l4span * P], bf16)
            nc.sync.dma_start(head, fm[:, :XOFF + CH])

            def xop_on(eng, out_ap, in_ap, bcol, relu):
                if eng == "act":
                    return nc.scalar.activation(
                        out_ap, in_ap, Act.Relu if relu else Act.Identity,
                        bias=bcol, scale=1.0,
                    )
                e = nc.vector if eng == "dve" else nc.gpsimd
                if relu:
                    return e.tensor_scalar(out_ap, in_ap, bcol, 0.0,
                                           Alu.add, Alu.max)
                return e.tensor_scalar(out_ap, in_ap, bcol, None, Alu.add)

            hcur = {}
            hl4 = {}
            pending_mm4 = []
            for gi, scs in enumerate(sgroups):
                # input DMA: first group loads in pieces behind the packed
                # head; later groups as one batched DMA each.
                def xdma(scs_):
                    xt = datap.tile([P, len(scs_) * CH], bf16, tag="x")
                    nc.sync.dma_start(
                        xt,
                        fm[:, XOFF + scs_[0] * CH:XOFF + (scs_[0] + len(scs_)) * CH])
                    for i, s in enumerate(scs_):
                        hcur[s] = xt[:, i * CH:(i + 1) * CH]

                if scs[0] == 0:
                    hcur[0] = head[:, XOFF:]
                    base = 1
                    for bi, blen in enumerate(cfg.get("g0split", [2, 4])):
                        xt = datap.tile([P, blen * CH], bf16, tag="x")
                        nc.sync.dma_start(
                            xt,
                            fm[:, XOFF + base * CH:XOFF + (base + blen) * CH])
                        for i in range(blen):
                            hcur[base + i] = xt[:, i * CH:(i + 1) * CH]
                        base += blen
                    assert base == len(scs), (base, len(scs))
                    # prefetch groups 1-2 ahead of the (large) w4 transfers
                    w4cut = min(NS, l4span) * P
                    if len(sgroups) > 1:
                        xdma(sgroups[1])
                    nc.sync.dma_start(w4tile[:, :w4cut], wst4[:, :w4cut])
                    if len(sgroups) > 2:
                        xdma(sgroups[2])
                    if w4cut < l4span * P:
                        nc.sync.dma_start(
                            w4tile[:, w4cut:], wst4[:, w4cut:])
                elif gi >= 3:
                    xdma(scs)

                def emit_mm4(mm4_scs):
                    # layer 4: sc s accumulates into bank 7 at rows 4j+m
                    # (j = position in its evac span); each span evacuates
                    # with ONE [4*span, 512] op + one output DMA.
                    for s in mm4_scs:
                        hbase, jj, slen, is_last = span_of_sc[s]
                        mm4 = nc.tensor.matmul(
                            l4ps, w4tile[:, jj * P:(jj + 1) * P],
                            hl4[s],
                            start=(jj == 0), stop=is_last,
                        )
                        NAME_INFO[mm4.ins.name] = (s, "mm4")
                        if is_last:
                            nrow = RG * slen
                            ob = outsp.tile([RG * l4span, CH], bf16,
                                            tag="ob")
                            span_idx = [sp[0] for sp in espans].index(hbase)
                            if span_idx in cfg.get("finsplit_spans", ()):
                                h = CH // 2
                                f1 = xop_on("act", ob[:nrow, :h],
                                            l4ps[:nrow, :h],
                                            btile[:nrow, 3:4], relu=False)
                                f2 = xop_on("dve", ob[:nrow, h:],
                                            l4ps[:nrow, h:],
                                            btile[:nrow, 3:4], relu=False)
                                NAME_INFO[f1.ins.name] = (hbase, "final.a")
                                NAME_INFO[f2.ins.name] = (hbase, "final.d")
                            elif cfg.get("finsplit"):
                                # column-split the evac across both engines
                                # so the tail-critical latency halves
                                h = CH // 2
                                f1 = xop_on("act", ob[:nrow, :h],
                                            l4ps[:nrow, :h],
                                            btile[:nrow, 3:4], relu=False)
                                f2 = xop_on("dve", ob[:nrow, h:],
                                            l4ps[:nrow, h:],
                                            btile[:nrow, 3:4], relu=False)
                                NAME_INFO[f1.ins.name] = (hbase, "final.a")
                                NAME_INFO[f2.ins.name] = (hbase, "final.d")
                            else:
                                fin = xop_on(fin_cfg, ob[:nrow, :],
                                             l4ps[:nrow, :],
                                             btile[:nrow, 3:4], relu=False)
                                NAME_INFO[fin.ins.name] = (
                                    hbase, f"final.{fin_cfg}")
                            # ONE DMA per span: SBUF side stays a plain
                            # single-partition-dim [4*slen, 512]; the
                            # (j, m, n) permutation lives on the DRAM side.
                            dmao = nc.sync.dma_start(
                                out_r[:, hbase:hbase + slen, :].rearrange(
                                    "m k n -> k m n"),
                                ob[:nrow, :],
                            )
                            NAME_INFO[dmao.ins.name] = (hbase, "dma_out")

                rowpat = pat[gi % len(pat)]
                ms = cfg.get("mm4split")
                mm4plan = cfg.get("mm4plan")
                m4e = cfg.get("mm4every", 1)
                hold_mm4 = (gi % m4e) != 0 and gi != len(sgroups) - 1
                weave = cfg.get("mm4weave")
                for layer in range(3):
                    for j, s in enumerate(scs):
                        ps = psum[:, (s % NS) * CH:((s % NS) + 1) * CH]
                        mm = nc.tensor.matmul(
                            ps, wtile[:, layer * P:(layer + 1) * P],
                            hcur[s], start=True, stop=True,
                        )
                        NAME_INFO[mm.ins.name] = (s, f"mm{layer}")
                        if weave and layer == 0 and pending_mm4 \
                                and not hold_mm4:
                            emit_mm4(pending_mm4[:1])
                            pending_mm4 = pending_mm4[1:]
                    if hold_mm4 or weave:
                        pass
                    elif mm4plan is not None:
                        n = mm4plan[layer] if layer < len(mm4plan) else 0
                        if n and pending_mm4:
                            emit_mm4(pending_mm4[:n])
                            pending_mm4 = pending_mm4[n:]
                        if layer == 2 and len(mm4plan) > 3 and pending_mm4:
                            pass  # remainder rides into the next group
                    elif ms and pending_mm4 and layer in (0, 1):
                        part = (pending_mm4[:ms] if layer == 0
                                else pending_mm4[ms:])
                        if layer == 1:
                            pending_mm4 = []
                        emit_mm4(part)
                    elif (not ms) and \
                            layer == cfg.get('mm4slot', 1) and pending_mm4:
                        emit_mm4(pending_mm4)
                        pending_mm4 = []
                    bcol = btile[:, layer:layer + 1]
                    g0row = cfg.get("g0rowspec")
                    if g0row is not None and gi == 0:
                        row = g0row[layer]
                    elif rowspec is not None:
                        row = rowspec[gi % len(rowspec)][layer]
                    else:
                        row = [(c0, clen, rowpat[layer][ci])
                               for ci, (c0, clen) in enumerate(cohorts)]
                    for ci, (c0, clen, eng) in enumerate(row):
                        if c0 >= len(scs):
                            continue
                        cl = min(clen, len(scs) - c0)
                        b0 = scs[c0] % NS
                        assert b0 + cl <= NS, (gi, c0, cl, b0)
                        ps = psum[:, b0 * CH:(b0 + cl) * CH]
                        hn = actp.tile([P, cl * CH], bf16,
                                       tag=f"h{layer}c{ci}")
                        xop = xop_on(eng, hn[:, :], ps, bcol, relu=True)
                        NAME_INFO[xop.ins.name] = (
                            scs[c0], f"relu{layer}.c{ci}.{eng}")
                        for i in range(cl):
                            hcur[scs[c0 + i]] = hn[:, i * CH:(i + 1) * CH]

                for s in scs:
                    hl4[s] = hcur[s]
                pending_mm4 = pending_mm4 + list(scs)

            if pending_mm4:
                emit_mm4(pending_mm4)

    nc.compile()
    return nc


def _blockdiag4(wT):
    """[32, 32] -> [128, 128] block-diagonal with 4 copies."""
    out = np.zeros((P, P), dtype=np.float32)
    for b in range(RG):
        out[32 * b:32 * b + 32, 32 * b:32 * b + 32] = wT
    return out


def _prep_host_inputs(z, w1, b1, w2, b2, w3, b3, wl, bl, cfg=None):
    """Fold z into the layer-1 bias and build the device weight layouts."""
    import ml_dtypes

    if cfg is None:
        cfg = DEFAULT_CFG
    espans_cfg = cfg.get("espans", [26, 26, 2])
    if sum(espans_cfg) != NSC:
        espans_cfg = [min(NS, NSC - g) for g in range(0, NSC, NS)]
    l4span = max(espans_cfg)

    f32 = np.float32
    b1e = (b1 + w1[:, C:] @ z[0]).astype(f32)          # [32]

    # w4 block j: L4-bank row 4j+m <- wl . (pixel-block m of span-member
    # j's sc) - j-major, matching the output DMA's (j, m, n) iteration.
    w4 = np.zeros((P, l4span * P), dtype=f32)
    for j in range(l4span):
        for m in range(RG):
            w4[32 * m:32 * m + 32, j * P + RG * j + m] = wl[0, :]

    wst3 = np.concatenate(
        [
            _blockdiag4(w1[:, :C].T),
            _blockdiag4(w2.T),
            _blockdiag4(w3.T),
        ],
        axis=1,
    ).astype(ml_dtypes.bfloat16)                        # [128, 384]

    bias = np.zeros((P, 4), dtype=f32)
    bias[:, 0] = np.tile(b1e, RG)
    bias[:, 1] = np.tile(b2.astype(f32), RG)
    bias[:, 2] = np.tile(b3.astype(f32), RG)
    bias[:, 3] = f32(bl[0])
    # bias packed as raw bf16-bit columns, appended to wst3 in fm_ext
    bias_bits = np.ascontiguousarray(bias).view(ml_dtypes.bfloat16)  # [128, 8]
    return np.concatenate([wst3, bias_bits], axis=1), \
        w4.astype(ml_dtypes.bfloat16)


def _restripe(shard):
    """[32, npix] channel-major shard -> [128, npix/4] (block, channel) rows."""
    npix = shard.shape[1]
    return np.ascontiguousarray(
        shard.reshape(C, RG, npix // RG).transpose(1, 0, 2).reshape(P, npix // RG)
    )


_NC_CACHE = {}
NAME_INFO = {}   # instruction name -> (sc, stage) for profiling


def _run(feature_map, z, w1, b1, w2, b2, w3, b3, wl, bl, **spmd_kwargs):
    import ml_dtypes
    from concourse.bass_utils import run_bass_kernel_spmd

    feature_map = np.asarray(feature_map, dtype=np.float32)
    z = np.asarray(z, dtype=np.float32)
    w1, b1 = np.asarray(w1, np.float32), np.asarray(b1, np.float32)
    w2, b2 = np.asarray(w2, np.float32), np.asarray(b2, np.float32)
    w3, b3 = np.asarray(w3, np.float32), np.asarray(b3, np.float32)
    wl, bl = np.asarray(wl, np.float32), np.asarray(bl, np.float32)

    wst3b, w4 = _prep_host_inputs(z, w1, b1, w2, b2, w3, b3, wl, bl)

    fm_flat = feature_map.reshape(C, VOL)
    in_maps = []
    for k in range(NCORES):
        shard = _restripe(fm_flat[:, k * NPIX:(k + 1) * NPIX]).astype(
            ml_dtypes.bfloat16
        )
        fm_ext = np.concatenate([wst3b, shard], axis=1)
        in_maps.append({"fm": fm_ext, "wst4": w4})

    if "nc" not in _NC_CACHE:
        _NC_CACHE["nc"] = _build_nc()
    nc = _NC_CACHE["nc"]

    res = run_bass_kernel_spmd(nc, in_maps, core_ids=list(range(NCORES)), **spmd_kwargs)
    out = np.empty((VOL,), dtype=np.float32)
    for k in range(NCORES):
        out[k * NPIX:(k + 1) * NPIX] = np.asarray(
            res.results[k]["out"]).astype(np.float32)
    return out.reshape(1, 1, 96, 96, 96), res


def kernel(feature_map, z, w1, b1, w2, b2, w3, b3, wl, bl):
    out, _ = _run(feature_map, z, w1, b1, w2, b2, w3, b3, wl, bl)
    return out


# revision 9
# speedup vs baseline: 1.0002x; 1.0002x over previous
"""Trainium2 Bass kernel for nn_FComb_79319456023150 (dense_cnn), v3.

Per-pixel MLP over a 96^3 volume: four 1x1x1 convs (38->32->32->32->1 channels
with relu between). z is batch-constant, so w1[:, 32:38] @ z folds into the
layer-1 bias and every layer becomes a K=32 channel GEMM.

Sharding: spatial (outermost X axis) across 8 cores, 110592 pixels each.
Weights/biases replicated.

Device layout per core: [128, 27648] = 4 pixel-blocks x 32 channels on
partitions, pixels on the free dim, bf16. Each layer is a BLOCK-DIAGONAL
[128, 128] matmul (4 copies of W^T), one full-array bf16 matmul per 512-col
sc applies the 32x32 GEMM to 4 pixel blocks at once.

Pipeline: 7 sc streams, one fp32 PSUM bank each; bank 7 is the L4
accumulator (rows 4j+m per 27-sc evac span). Relu+bias rides the mandatory
PSUM->SBUF crossing, which only Act and DVE may perform (GPSIMD has no PSUM
port), and in steady state PE (5973ns/7-sc group), Act (~5950) and DVE
(~6080) are all ~100% committed - the schedule is a measured three-way tie,
which is why crossing cohorts stay at [128,1024] pairs (every larger-cohort
variant loses >1us to latency cascades; measured via TimelineSim sweeps).

v3 structure (vs the previous 60.3us kernel):
 - Startup: the first DMA packs [w1|w2|w3] + the fp32 bias (bitcast to bf16
   bit-columns) + the first sc of pixels into ONE transfer (one HWDGE slot
   instead of three), group 0 loads sc-by-sc with single-sc crossing cohorts
   (pipeline ramps while DMA streams), and groups 1-2 prefetch ahead of the
   large w4 transfer. First real matmul at ~3.5us (DMA-latency floor).
 - The FIRST group is 5 scs wide (headgroup=5), so the DMA-starved ramp
   group is smaller and every later group is a full 7-wide phase-5 bank
   rotation (crossing cohorts index banks via scs[c0] mod 7). Worth ~0.9us.
 - A gpsimd-memset warm tile feeds (a) a dummy activation emitted first so
   the ACT table load happens at t~0 instead of stalling the first crossing
   (-1.3us), and (b) a burst of tiny warm-up matmuls so the PE p-state ramp
   is spent on dummy work while the first input DMA is in flight (-2.4us
   measured vs warm_n=0).
 - L4 evac spans are [27, 27] so each half ships its own output DMA; the
   second evac+DMA+sem chain is the unavoidable ~3.7us tail.
"""

import sys

import numpy as np

if "/opt/trn_rl_repo" not in sys.path:
    sys.path.insert(0, "/opt/trn_rl_repo")

C = 32          # channels per layer
P = 128         # SBUF/PSUM partitions
RG = 4          # pixel blocks stacked on the partition dim (128/32)
CH = 512        # sc width = one PSUM bank of fp32
VOL = 96 * 96 * 96                   # full volume
NCORES = 8
NPIX = VOL // NCORES                 # 110592 pixels per core
FREE = NPIX // RG                    # 27648 free-dim columns per core
NSC = FREE // CH                     # 54 scs per core
NS = 7                               # parallel sc streams (PSUM banks 0-6)
WCOLS = 3 * P                        # packed [w1|w2|w3] columns in fm_ext
assert FREE % CH == 0


# Group 0 runs its L0/L1 crossings as single-sc ops (alternating engines)
# so the pipeline ramps sc-by-sc while the input DMA is still streaming;
# L2 back to steady [1024] pairs. Found by TimelineSim search.
G0ROW_DEFAULT = [
    [(0, 1, "dve"), (1, 1, "act"), (2, 1, "dve"), (3, 1, "act"),
     (4, 1, "dve"), (5, 1, "act"), (6, 1, "dve")],
    [(0, 1, "act"), (1, 1, "dve"), (2, 1, "act"), (3, 1, "dve"),
     (4, 1, "act"), (5, 1, "dve"), (6, 1, "act")],
    [(0, 2, "act"), (2, 2, "dve"), (4, 2, "act"), (6, 1, "dve")],
]

DEFAULT_CFG = {
    "pat": [
        [("act", "dve", "act", "dve"), ("dve", "act", "dve", "act"),
         ("act", "dve", "act", "dve")],
    ],
    "fin": "act",
    "headgroup": 5,
    "g0split": [1, 1, 2],
    "cohorts": [(0, 2), (2, 2), (4, 2), (6, 1)],
    "mm4slot": 1,
    "mm4plan": [0, 5, 2],
    "xbufs": 6,
    "warm_n": 20,
    "warm_cols": 48,
    "early_act": True,
    "espans": [29, 25],
    "fin_spans": {"1": "dve"},
    "g0rowspec": G0ROW_DEFAULT,
}


def _build_nc(npix=NPIX, cfg=None):
    import concourse.mybir as mybir
    from concourse import bacc
    from concourse.tile import TileContext

    f32 = mybir.dt.float32
    bf16 = mybir.dt.bfloat16
    Alu = mybir.AluOpType
    Act = mybir.ActivationFunctionType

    if cfg is None:
        cfg = DEFAULT_CFG
    pat = cfg["pat"]
    fin_cfg = cfg.get("fin", "act")
    cohorts = cfg.get("cohorts", [(0, 2), (2, 2), (4, 2), (6, 1)])
    rowspec = cfg.get("rowspec")

    free = npix // RG
    nsc = free // CH
    assert free % CH == 0 and nsc >= 1
    espans_cfg = cfg.get("espans", [26, 26, 2])
    if sum(espans_cfg) != nsc:
        espans_cfg = [min(NS, nsc - g) for g in range(0, nsc, NS)]
    l4span = max(espans_cfg)

    nc = bacc.Bacc()
    # fm_ext: [w1|w2|w3] block-diag weights (384 cols), the fp32 bias packed
    # as 8 bf16-bit columns, then the pixel data.
    fm = nc.dram_tensor("fm", [P, WCOLS + 8 + free], bf16, kind="ExternalInput")
    wst4 = nc.dram_tensor("wst4", [P, l4span * P], bf16, kind="ExternalInput")
    out = nc.dram_tensor("out", [npix], bf16, kind="ExternalOutput")
    XOFF = WCOLS + 8

    # out[m*free + s*CH + n] viewed [m, s, n] for per-group stores
    out_r = out.rearrange("(m s n) -> m s n", m=RG, s=nsc, n=CH)

    hg = cfg.get("headgroup")
    if hg:
        sgroups = [list(range(hg))] + [
            list(range(g, min(g + NS, nsc))) for g in range(hg, nsc, NS)]
    else:
        sgroups = [list(range(g, min(g + NS, nsc)))
                   for g in range(0, nsc, NS)]
    ts = cfg.get("tailsplit")
    if ts and len(sgroups[-1]) > max(ts):
        last = sgroups.pop()
        i = 0
        for w in ts:
            sgroups.append(last[i:i + w])
            i += w
        assert i == len(last), (ts, len(last))

    # evac spans: consecutive sc runs flushed together from bank 7
    espans = []
    s0 = 0
    for w in espans_cfg:
        espans.append(list(range(s0, s0 + w)))
        s0 += w
    assert s0 == nsc
    span_of_sc = {}
    for scs in espans:
        for j, s in enumerate(scs):
            span_of_sc[s] = (scs[0], j, len(scs), s == scs[-1])
    assert all(4 * (len(scs)) <= P for scs in espans)

    with TileContext(nc) as tc:
        with (
            tc.tile_pool(name="const", bufs=1) as constp,
            tc.tile_pool(name="data", bufs=cfg.get("xbufs", 4)) as datap,
            tc.tile_pool(name="acts", bufs=cfg.get("hbufs", 2)) as actp,
            tc.tile_pool(name="outs", bufs=2) as outsp,
            tc.tile_pool(name="psb", bufs=1, space="PSUM") as psb,
        ):
            psum = psb.tile([P, 8 * CH], f32)
            l4ps = psum[:, NS * CH:(NS + 1) * CH]

            # Warm tile: memset on the idle Pool engine at t~0; feeds the
            # early dummy activation (pulls the ACT table load off the
            # first crossing's critical path) and the PE warm-up burst.
            wcols = cfg.get("warm_cols", 48)
            warm = constp.tile([P, max(wcols, 4)], bf16)
            wscr = constp.tile([P, 4], bf16)
            if cfg.get("warm_memset_eng", "pool") == "dve":
                nc.vector.memset(warm[:, :], 0)
            else:
                nc.gpsimd.memset(warm[:, :], 0)
            if cfg.get("early_act", True):
                ea = nc.scalar.activation(
                    wscr[:, 0:2], warm[:, 0:2], Act.Relu, bias=0.0, scale=1.0)
                NAME_INFO[ea.ins.name] = (-1, "early_act")
            for wi in range(cfg.get("warm_n", 0)):
                wm = nc.tensor.matmul(
                    l4ps[:wcols, :wcols], warm[:, :wcols], warm[:, :wcols],
                    start=True, stop=True)
                NAME_INFO[wm.ins.name] = (-1, f"warm{wi}")

            # head tile: [w1|w2|w3] + packed bias + first sc in ONE DMA.
            head = constp.tile([P, XOFF + CH], bf16)
            wtile = head[:, :WCOLS]
            btile = head[:, WCOLS:XOFF].bitcast(f32)
            w4tile = constp.tile([P, l4span * P], bf16)
            nc.sync.dma_start(head, fm[:, :XOFF + CH])

            def xop_on(eng, out_ap, in_ap, bcol, relu):
                if eng == "act":
                    return nc.scalar.activation(
                        out_ap, in_ap, Act.Relu if relu else Act.Identity,
                        bias=bcol, scale=1.0,
                    )
                e = nc.vector if eng == "dve" else nc.gpsimd
                if relu:
                    return e.tensor_scalar(out_ap, in_ap, bcol, 0.0,
                                           Alu.add, Alu.max)
                return e.tensor_scalar(out_ap, in_ap, bcol, None, Alu.add)

            hcur = {}
            hl4 = {}
            pending_mm4 = []
            for gi, scs in enumerate(sgroups):
                # input DMA: first group loads in pieces behind the packed
                # head; later groups as one batched DMA each.
                def xdma(scs_):
                    xt = datap.tile([P, len(scs_) * CH], bf16, tag="x")
                    nc.sync.dma_start(
                        xt,
                        fm[:, XOFF + scs_[0] * CH:XOFF + (scs_[0] + len(scs_)) * CH])
                    for i, s in enumerate(scs_):
                        hcur[s] = xt[:, i * CH:(i + 1) * CH]

                if scs[0] == 0:
                    hcur[0] = head[:, XOFF:]
                    base = 1
                    for bi, blen in enumerate(cfg.get("g0split", [2, 4])):
                        xt = datap.tile([P, blen * CH], bf16, tag="x")
                        nc.sync.dma_start(
                            xt,
                            fm[:, XOFF + base * CH:XOFF + (base + blen) * CH])
                        for i in range(blen):
                            hcur[base + i] = xt[:, i * CH:(i + 1) * CH]
                        base += blen
                    assert base == len(scs), (base, len(scs))
                    # prefetch groups 1-2 ahead of the (large) w4 transfers
                    w4cut = min(NS, l4span) * P
                    if len(sgroups) > 1:
                        xdma(sgroups[1])
                    nc.sync.dma_start(w4tile[:, :w4cut], wst4[:, :w4cut])
                    if len(sgroups) > 2:
                        xdma(sgroups[2])
                    if w4cut < l4span * P:
                        nc.sync.dma_start(
                            w4tile[:, w4cut:], wst4[:, w4cut:])
                elif gi >= 3:
                    xdma(scs)

                def emit_mm4(mm4_scs):
                    # layer 4: sc s accumulates into bank 7 at rows 4j+m
                    # (j = position in its evac span); each span evacuates
                    # with ONE [4*span, 512] op + one output DMA.
                    for s in mm4_scs:
                        hbase, jj, slen, is_last = span_of_sc[s]
                        mm4 = nc.tensor.matmul(
                            l4ps, w4tile[:, jj * P:(jj + 1) * P],
                            hl4[s],
                            start=(jj == 0), stop=is_last,
                        )
                        NAME_INFO[mm4.ins.name] = (s, "mm4")
                        if is_last:
                            nrow = RG * slen
                            ob = outsp.tile([RG * l4span, CH], bf16,
                                            tag="ob")
                            span_idx = [sp[0] for sp in espans].index(hbase)
                            if span_idx in cfg.get("finsplit_spans", ()):
                                h = CH // 2
                                f1 = xop_on("act", ob[:nrow, :h],
                                            l4ps[:nrow, :h],
                                            btile[:nrow, 3:4], relu=False)
                                f2 = xop_on("dve", ob[:nrow, h:],
                                            l4ps[:nrow, h:],
                                            btile[:nrow, 3:4], relu=False)
                                NAME_INFO[f1.ins.name] = (hbase, "final.a")
                                NAME_INFO[f2.ins.name] = (hbase, "final.d")
                            elif str(span_idx) in cfg.get("fin_spans", {}):
                                fe = cfg["fin_spans"][str(span_idx)]
                                fin = xop_on(fe, ob[:nrow, :],
                                             l4ps[:nrow, :],
                                             btile[:nrow, 3:4], relu=False)
                                NAME_INFO[fin.ins.name] = (
                                    hbase, f"final.{fe}")
                            elif cfg.get("finsplit"):
                                # column-split the evac across both engines
                                # so the tail-critical latency halves
                                h = CH // 2
                                f1 = xop_on("act", ob[:nrow, :h],
                                            l4ps[:nrow, :h],
                                            btile[:nrow, 3:4], relu=False)
                                f2 = xop_on("dve", ob[:nrow, h:],
                                            l4ps[:nrow, h:],
                                            btile[:nrow, 3:4], relu=False)
                                NAME_INFO[f1.ins.name] = (hbase, "final.a")
                                NAME_INFO[f2.ins.name] = (hbase, "final.d")
                            else:
                                fin = xop_on(fin_cfg, ob[:nrow, :],
                                             l4ps[:nrow, :],
                                             btile[:nrow, 3:4], relu=False)
                                NAME_INFO[fin.ins.name] = (
                                    hbase, f"final.{fin_cfg}")
                            # ONE DMA per span: SBUF side stays a plain
                            # single-partition-dim [4*slen, 512]; the
                            # (j, m, n) permutation lives on the DRAM side.
                            dmao = nc.sync.dma_start(
                                out_r[:, hbase:hbase + slen, :].rearrange(
                                    "m k n -> k m n"),
                                ob[:nrow, :],
                            )
                            NAME_INFO[dmao.ins.name] = (hbase, "dma_out")

                rowpat = pat[gi % len(pat)]
                ms = cfg.get("mm4split")
                mm4plan = cfg.get("mm4plan")
                m4e = cfg.get("mm4every", 1)
                hold_mm4 = (gi % m4e) != 0 and gi != len(sgroups) - 1
                weave = cfg.get("mm4weave")
                for layer in range(3):
                    for j, s in enumerate(scs):
                        ps = psum[:, (s % NS) * CH:((s % NS) + 1) * CH]
                        mm = nc.tensor.matmul(
                            ps, wtile[:, layer * P:(layer + 1) * P],
                            hcur[s], start=True, stop=True,
                        )
                        NAME_INFO[mm.ins.name] = (s, f"mm{layer}")
                        if weave and layer == 0 and pending_mm4 \
                                and not hold_mm4:
                            emit_mm4(pending_mm4[:1])
                            pending_mm4 = pending_mm4[1:]
                    if hold_mm4 or weave:
                        pass
                    elif mm4plan is not None:
                        n = mm4plan[layer] if layer < len(mm4plan) else 0
                        if n and pending_mm4:
                            emit_mm4(pending_mm4[:n])
                            pending_mm4 = pending_mm4[n:]
                        if layer == 2 and len(mm4plan) > 3 and pending_mm4:
                            pass  # remainder rides into the next group
                    elif ms and pending_mm4 and layer in (0, 1):
                        part = (pending_mm4[:ms] if layer == 0
                                else pending_mm4[ms:])
                        if layer == 1:
                            pending_mm4 = []
                        emit_mm4(part)
                    elif (not ms) and \
                            layer == cfg.get('mm4slot', 1) and pending_mm4:
                        emit_mm4(pending_mm4)
                        pending_mm4 = []
                    bcol = btile[:, layer:layer + 1]
                    g0row = cfg.get("g0rowspec")
                    if g0row is not None and gi == 0:
                        row = g0row[layer]
                    elif rowspec is not None:
                        row = rowspec[gi % len(rowspec)][layer]
                    else:
                        row = [(c0, clen, rowpat[layer][ci])
                               for ci, (c0, clen) in enumerate(cohorts)]
                    for ci, (c0, clen, eng) in enumerate(row):
                        if c0 >= len(scs):
                            continue
                        cl = min(clen, len(scs) - c0)
                        b0 = scs[c0] % NS
                        assert b0 + cl <= NS, (gi, c0, cl, b0)
                        ps = psum[:, b0 * CH:(b0 + cl) * CH]
                        hn = actp.tile([P, cl * CH], bf16,
                                       tag=f"h{layer}c{ci}")
                        xop = xop_on(eng, hn[:, :], ps, bcol, relu=True)
                        NAME_INFO[xop.ins.name] = (
                            scs[c0], f"relu{layer}.c{ci}.{eng}")
                        for i in range(cl):
                            hcur[scs[c0 + i]] = hn[:, i * CH:(i + 1) * CH]

                for s in scs:
                    hl4[s] = hcur[s]
                pending_mm4 = pending_mm4 + list(scs)

            if pending_mm4:
                emit_mm4(pending_mm4)

    nc.compile()
    return nc


def _blockdiag4(wT):
    """[32, 32] -> [128, 128] block-diagonal with 4 copies."""
    out = np.zeros((P, P), dtype=np.float32)
    for b in range(RG):
        out[32 * b:32 * b + 32, 32 * b:32 * b + 32] = wT
    return out


def _prep_host_inputs(z, w1, b1, w2, b2, w3, b3, wl, bl, cfg=None):
    """Fold z into the layer-1 bias and build the device weight layouts."""
    import ml_dtypes

    if cfg is None:
        cfg = DEFAULT_CFG
    espans_cfg = cfg.get("espans", [26, 26, 2])
    if sum(espans_cfg) != NSC:
        espans_cfg = [min(NS, NSC - g) for g in range(0, NSC, NS)]
    l4span = max(espans_cfg)

    f32 = np.float32
    b1e = (b1 + w1[:, C:] @ z[0]).astype(f32)          # [32]

    # w4 block j: L4-bank row 4j+m <- wl . (pixel-block m of span-member
    # j's sc) - j-major, matching the output DMA's (j, m, n) iteration.
    w4 = np.zeros((P, l4span * P), dtype=f32)
    for j in range(l4span):
        for m in range(RG):
            w4[32 * m:32 * m + 32, j * P + RG * j + m] = wl[0, :]

    wst3 = np.concatenate(
        [
            _blockdiag4(w1[:, :C].T),
            _blockdiag4(w2.T),
            _blockdiag4(w3.T),
        ],
        axis=1,
    ).astype(ml_dtypes.bfloat16)                        # [128, 384]

    bias = np.zeros((P, 4), dtype=f32)
    bias[:, 0] = np.tile(b1e, RG)
    bias[:, 1] = np.tile(b2.astype(f32), RG)
    bias[:, 2] = np.tile(b3.astype(f32), RG)
    bias[:, 3] = f32(bl[0])
    # bias packed as raw bf16-bit columns, appended to wst3 in fm_ext
    bias_bits = np.ascontiguousarray(bias).view(ml_dtypes.bfloat16)  # [128, 8]
    return np.concatenate([wst3, bias_bits], axis=1), \
        w4.astype(ml_dtypes.bfloat16)


def _restripe(shard):
    """[32, npix] channel-major shard -> [128, npix/4] (block, channel) rows."""
    npix = shard.shape[1]
    return np.ascontiguousarray(
        shard.reshape(C, RG, npix // RG).transpose(1, 0, 2).reshape(P, npix // RG)
    )


_NC_CACHE = {}
NAME_INFO = {}   # instruction name -> (sc, stage) for profiling


def _run(feature_map, z, w1, b1, w2, b2, w3, b3, wl, bl, **spmd_kwargs):
    import ml_dtypes
    from concourse.bass_utils import run_bass_kernel_spmd

    feature_map = np.asarray(feature_map, dtype=np.float32)
    z = np.asarray(z, dtype=np.float32)
    w1, b1 = np.asarray(w1, np.float32), np.asarray(b1, np.float32)
    w2, b2 = np.asarray(w2, np.float32), np.asarray(b2, np.float32)
    w3, b3 = np.asarray(w3, np.float32), np.asarray(b3, np.float32)
    wl, bl = np.asarray(wl, np.float32), np.asarray(bl, np.float32)

    wst3b, w4 = _prep_host_inputs(z, w1, b1, w2, b2, w3, b3, wl, bl)

    fm_flat = feature_map.reshape(C, VOL)
    in_maps = []
    for k in range(NCORES):
        shard = _restripe(fm_flat[:, k * NPIX:(k + 1) * NPIX]).astype(
            ml_dtypes.bfloat16
        )
        fm_ext = np.concatenate([wst3b, shard], axis=1)
        in_maps.append({"fm": fm_ext, "wst4": w4})

    if "nc" not in _NC_CACHE:
        _NC_CACHE["nc"] = _build_nc()
    nc = _NC_CACHE["nc"]

    res = run_bass_kernel_spmd(nc, in_maps, core_ids=list(range(NCORES)), **spmd_kwargs)
    out = np.empty((VOL,), dtype=np.float32)
    for k in range(NCORES):
        out[k * NPIX:(k + 1) * NPIX] = np.asarray(
            res.results[k]["out"]).astype(np.float32)
    return out.reshape(1, 1, 96, 96, 96), res


def kernel(feature_map, z, w1, b1, w2, b2, w3, b3, wl, bl):
    out, _ = _run(feature_map, z, w1, b1, w2, b2, w3, b3, wl, bl)
    return out
